# revision 18
# baseline (speedup 1.0000x reference)
"""Causal multi-head attention (B=1, S=4096, D=768, H=12, d_head=64) on 8
Trainium2 NeuronCores.

Sharding: exact 1.5 heads per core. Slot A = head c (c = core id 0..7), full
causal attention over all 4096 queries. Slot B = head 8 + c//2 restricted to
query tokens of parity c%2 (2048 alternate tokens, full key range), so the 4
remaining heads are each split across two cores by query parity with zero
duplicated work and a uniform SPMD program (the parity lives in the data:
host-gathered xB rows and a parity-dependent boundary mask).

All matmul operands are bf16 (PSUM accumulation stays f32); the host supplies
x already transposed (and parity-gathered for slot B), so the device does no
x transposes at all. Per query tile the kernel interleaves next-tile QKV
projections and previous-tile out-projections into the attention block loop to
keep the PE busy (and at full clock) while the Scalar engine runs the exps.
Softmax denominators come free via ones-columns appended to V; normalization
uses reciprocal_approx_fast on DVE; out-proj PSUM->SBUF copies run on GpSimd.
Partial outputs are written bf16; the host sums them (the all-reduce of the
row-parallel out projection) and adds b_out.
"""

import sys

sys.path.insert(0, "/opt/trn_rl_repo")

from collections import deque

import ml_dtypes
import numpy as np

import concourse.bass as bass
import concourse.tile as tile
from concourse import bacc, mybir
from concourse.bass_utils import run_bass_kernel_spmd

S = 4096
D = 768
HD = 64
P = 128
KC = D // P  # 6 contraction chunks for the projections
NT = 8  # 512-token query tiles
NEG = -1e30

F32 = mybir.dt.float32
BF16 = mybir.dt.bfloat16
AF = mybir.ActivationFunctionType
ADD = mybir.AluOpType.add
MULT = mybir.AluOpType.mult

_CACHED_NC = None


def build_nc():
    nc = bacc.Bacc("TRN2", target_bir_lowering=False, debug=False, num_devices=8)

    xt_d = nc.declare_dram_parameter("xt", [8 * P, KC, 512], BF16, isOutput=False)
    xb_d = nc.declare_dram_parameter("xb", [4 * P, KC, 512], BF16, isOutput=False)
    wq_d = nc.declare_dram_parameter("wq", [P, KC, P], BF16, isOutput=False)
    wk_d = nc.declare_dram_parameter("wk", [P, KC, P], BF16, isOutput=False)
    wv_d = nc.declare_dram_parameter("wv", [P, KC, P], BF16, isOutput=False)
    wo_d = nc.declare_dram_parameter("wo", [P, D], BF16, isOutput=False)
    ma_d = nc.declare_dram_parameter("ma", [P, P], F32, isOutput=False)
    mb_d = nc.declare_dram_parameter("mb", [P, HD], F32, isOutput=False)
    id_d = nc.declare_dram_parameter("ident", [P, P], BF16, isOutput=False)
    outa_d = nc.declare_dram_parameter("outA", [S, D], BF16, isOutput=True)
    outb_d = nc.declare_dram_parameter("outB", [S // 2, D], BF16, isOutput=True)

    with tile.TileContext(nc) as tc:
        with (
            tc.tile_pool(name="const", bufs=1) as const,
            tc.tile_pool(name="big", bufs=1) as big,
            tc.tile_pool(name="pt", bufs=6) as ptp,
            tc.tile_pool(name="vt", bufs=2) as vtp,
            tc.tile_pool(name="osb", bufs=3) as osbp,
            tc.tile_pool(name="sm", bufs=2) as sm,
            tc.tile_pool(name="sc2", bufs=2, space="PSUM") as sc2p,
            tc.tile_pool(name="ps", bufs=1, space="PSUM") as ps,
            tc.tile_pool(name="ctxA", bufs=1, space="PSUM") as ctxAp,
            tc.tile_pool(name="ctxB", bufs=1, space="PSUM") as ctxBp,
            tc.tile_pool(name="tpp", bufs=1, space="PSUM") as tpp,
        ):
            # ---- constants ----
            # Split each load 4 ways by partition: one queue processes
            # descriptors serially (~150ns each), so a 128-partition DMA on a
            # single queue takes ~19us; 4 queues cut that to ~5us.
            def dma4(dst, src):
                for q4 in range(4):
                    nc.sync.dma_start(dst[32 * q4 : 32 * (q4 + 1)],
                                      src[32 * q4 : 32 * (q4 + 1)])

            identb = const.tile([P, P], BF16)
            dma4(identb, id_d)
            ma_s = const.tile([P, P], F32)
            dma4(ma_s, ma_d)
            mb_s = const.tile([P, HD], F32)
            dma4(mb_s, mb_d)
            wq_s = const.tile([P, KC, P], BF16)
            dma4(wq_s, wq_d)
            wk_s = const.tile([P, KC, P], BF16)
            dma4(wk_s, wk_d)
            wv_s = const.tile([P, KC, P], BF16)
            dma4(wv_s, wv_d)
            wo_s = const.tile([P, D], BF16)
            dma4(wo_s, wo_d)

            # ---- persistent activations ----
            xTs = big.tile([P, NT, KC, 512], BF16)  # x^T, group-major
            xBs = big.tile([P, 4, KC, 512], BF16)  # x^T of slot-B tokens
            qT = big.tile([P, S], BF16)  # rows 0:64 qA^T, 64:128 qB^T (cols 0:2048)
            k2 = big.tile([P, S], BF16)  # rows 0:64 kA^T, 64:128 kB^T
            # v natural per 128-key block: cols 0:64 vA, 64 ones, 66:130 vB,
            # 130 ones (65/131 unused)
            vNat = big.tile([P, S // P, 132], BF16)
            cT = big.tile([P, S], BF16)  # rows 0:64 ctxA^T, 64:128 ctxB^T

            for t in range(NT):
                for q4 in range(4):
                    nc.sync.dma_start(
                        xTs[32 * q4 : 32 * (q4 + 1), t, :, :],
                        xt_d[P * t + 32 * q4 : P * t + 32 * (q4 + 1), :, :],
                    )
            for g in range(4):
                for q4 in range(4):
                    nc.sync.dma_start(
                        xBs[32 * q4 : 32 * (q4 + 1), g, :, :],
                        xb_d[P * g + 32 * q4 : P * g + 32 * (q4 + 1), :, :],
                    )

            nc.gpsimd.memset(vNat[:, :, 64], 1.0)
            nc.gpsimd.memset(vNat[:, :, 130], 1.0)

            # ---- PE warmup: ramp the clock while DMAs stream in ----
            for _ in range(24):
                wps = ps.tile([P, 512], F32, name="ps", tag="ps")
                nc.tensor.matmul(
                    wps[:, 0:P], identb[:], identb[:], start=True, stop=True
                )

            # ---- projection pieces for tile group t ----
            def mk_projK(t):
                def f():
                    pp = ps.tile([P, 512], F32, name="ps", tag="ps")
                    for c in range(KC):
                        nc.tensor.matmul(
                            pp[:],
                            wk_s[:, c, :],
                            xTs[:, t, c, :],
                            start=(c == 0),
                            stop=(c == KC - 1),
                        )
                    nc.vector.tensor_copy(k2[:, 512 * t : 512 * (t + 1)], pp[:])

                return f

            def mk_projV(t):
                def f():
                    pp = ps.tile([P, 512], F32, name="ps", tag="ps")
                    for c in range(KC):
                        nc.tensor.matmul(
                            pp[:],
                            wv_s[:, c, :],
                            xTs[:, t, c, :],
                            start=(c == 0),
                            stop=(c == KC - 1),
                        )
                    vt_t = vtp.tile([P, 512], BF16, name="vt")
                    nc.vector.tensor_copy(vt_t[:], pp[:])
                    f.vt = vt_t

                return f

            def mk_projQ(t):
                def f():
                    pp = ps.tile([P, 512], F32, name="ps", tag="ps")
                    for c in range(KC):
                        nc.tensor.matmul(
                            pp[0:HD, :],
                            wq_s[:, c, 0:HD],
                            xTs[:, t, c, :],
                            start=(c == 0),
                            stop=(c == KC - 1),
                        )
                    if t % 2 == 0:
                        g = t // 2
                        for c in range(KC):
                            nc.tensor.matmul(
                                pp[HD:P, :],
                                wq_s[:, c, HD:P],
                                xBs[:, g, c, :],
                                start=(c == 0),
                                stop=(c == KC - 1),
                            )
                    nc.vector.tensor_copy(
                        qT[0:HD, 512 * t : 512 * (t + 1)], pp[0:HD, :]
                    )
                    if t % 2 == 0:
                        g = t // 2
                        nc.vector.tensor_copy(
                            qT[HD:P, 512 * g : 512 * (g + 1)], pp[HD:P, :]
                        )

                return f

            def mk_transV(t, projv):
                def f():
                    tp = tpp.tile([P, 4, P], BF16, name="tp")
                    for b in range(4):
                        nc.tensor.transpose(
                            tp[:, b, :],
                            projv.vt[:, P * b : P * (b + 1)],
                            identb[:],
                        )
                    nc.vector.tensor_copy(
                        vNat[:, 4 * t : 4 * t + 4, 0:HD], tp[:, :, 0:HD]
                    )
                    nc.vector.tensor_copy(
                        vNat[:, 4 * t : 4 * t + 4, 66:130], tp[:, :, HD:P]
                    )

                return f

            def proj_pieces(t):
                pv = mk_projV(t)
                return [mk_projK(t), pv, mk_projQ(t), mk_transV(t, pv)]

            # ---- out-projection piece for one 128-row block ----
            def mk_outp(st, is_b):
                def f():
                    osb_t = osbp.tile([P, D], BF16, name="osb")
                    crow = cT[HD:P, :] if is_b else cT[0:HD, :]
                    wrow = wo_s[HD:P, :] if is_b else wo_s[0:HD, :]
                    for h in range(2):
                        po = ps.tile([P, 512], F32, name="ps", tag="ps")
                        nc.tensor.matmul(
                            po[:, 0:384],
                            crow[:, P * st : P * (st + 1)],
                            wrow[:, 384 * h : 384 * (h + 1)],
                            start=True,
                            stop=True,
                        )
                        nc.vector.tensor_copy(
                            osb_t[:, 384 * h : 384 * (h + 1)], po[:, 0:384]
                        )
                    dst = outb_d if is_b else outa_d
                    nc.sync.dma_start(dst[P * st : P * (st + 1), :], osb_t[:])

                return f

            def outpA_pieces(t):
                return [mk_outp(4 * t + i, False) for i in range(4)]

            def outpB_pieces(T):
                return [mk_outp(4 * T + i, True) for i in range(4)]

            # ---- attention for tile t; pops bg pieces into PE slack ----
            # Slot B runs 512-wide query tiles (B-tile T spans A-tiles 2T and
            # 2T+1): key blocks 0..8T+3 during the even tile, the 4 diagonal
            # blocks during the odd tile.
            def attn(t, bgP, bgO, bstate):
                T = t // 2
                nkb = 4 * (t + 1)
                ctxA_t = ctxAp.tile([P, 512], F32, name="ctxA")
                if t % 2 == 0:
                    bstate["ctxB"] = ctxBp.tile([P, 512], F32, name="ctxB")
                    bstate["pend"] = None
                    b_list = list(range(0, 8 * T + 4))
                else:
                    b_list = list(range(8 * T + 4, 8 * T + 8))
                ctxB_t = bstate["ctxB"]
                pend_A = None  # (pA, r0A, kb)

                def issue_ctxA(p, last):
                    pA, r0A, kb = p
                    nc.tensor.matmul(
                        ctxA_t[0:65, r0A:512],
                        vNat[:, kb, 0:65],
                        pA[:, r0A:512],
                        start=(kb == 0),
                        stop=last,
                    )

                def issue_ctxB(p, last):
                    pB, r0B, kb = p
                    nc.tensor.matmul(
                        ctxB_t[0:65, r0B:512],
                        vNat[:, kb, 66:131],
                        pB[:, 512 + r0B : 1024],
                        start=(kb == 0),
                        stop=last,
                    )

                bi = 0
                for i in range(nkb):
                    kb = i
                    if (
                        t % 2 == 1
                        and bi == len(b_list)
                        and bstate["pend"] is not None
                    ):
                        # B-tile T complete: normalize now and queue its
                        # out-proj into the remaining PE slack.
                        issue_ctxB(bstate["pend"], True)
                        bstate["pend"] = None
                        normalize_B(T, bstate)
                        bgO.extend(outpB_pieces(T))
                    d = kb - 4 * t  # >= 0 on diagonal blocks
                    # --- A and B scores share one 2-bank PSUM tile so a
                    # single exp instruction covers both (A in [0:512], B in
                    # [512:1024]; the gap on diagonal iterations holds stale
                    # bounded scores, exp'd but never read by ctx).
                    r0A = P * d if d >= 0 else 0
                    sc = sc2p.tile([P, 1024], F32, name="sc2")
                    nc.tensor.matmul(
                        sc[:, r0A:512],
                        k2[0:HD, P * kb : P * (kb + 1)],
                        qT[0:HD, 512 * t + r0A : 512 * (t + 1)],
                        start=True,
                        stop=True,
                        skip_group_check=True,
                    )
                    if d >= 0:
                        nc.vector.tensor_tensor(
                            sc[:, r0A : r0A + P],
                            sc[:, r0A : r0A + P],
                            ma_s[:],
                            ADD,
                        )
                    has_b = bi < len(b_list)
                    pt_t = ptp.tile([P, 1024], BF16, name="pt", tag="pt")
                    if has_b:
                        kbb = b_list[bi]
                        bi += 1
                        dB = kbb - 8 * T
                        r0B = HD * dB if dB >= 0 else 0
                        nc.tensor.matmul(
                            sc[:, 512 + r0B : 1024],
                            k2[HD:P, P * kbb : P * (kbb + 1)],
                            qT[HD:P, 512 * T + r0B : 512 * (T + 1)],
                            start=True,
                            stop=True,
                            skip_group_check=True,
                        )
                        if dB >= 0:
                            nc.vector.tensor_tensor(
                                sc[:, 512 + r0B : 512 + r0B + HD],
                                sc[:, 512 + r0B : 512 + r0B + HD],
                                mb_s[:],
                                ADD,
                            )
                        nc.scalar.activation(
                            pt_t[:, r0A:1024], sc[:, r0A:1024],
                            AF.Exp, scale=0.125,
                        )
                        if bstate["pend"] is not None:
                            issue_ctxB(bstate["pend"], False)
                        bstate["pend"] = (pt_t, r0B, kbb)
                    else:
                        nc.scalar.activation(
                            pt_t[:, r0A:512], sc[:, r0A:512],
                            AF.Exp, scale=0.125,
                        )
                    # --- lagged ctx for slot A
                    if pend_A is not None:
                        issue_ctxA(pend_A, False)
                    pend_A = (pt_t, r0A, kb)
                    # --- background pieces: proj must finish this tile;
                    # out-proj pieces carry over without burst drains.
                    rem = nkb - i
                    if bgP:
                        npop = min(len(bgP), max(1, -(-len(bgP) // rem)))
                        for _ in range(npop):
                            bgP.popleft()()
                    elif bgO and i % 2 == 0:
                        bgO.popleft()()
                issue_ctxA(pend_A, True)
                if t % 2 == 1 and bstate["pend"] is not None:
                    issue_ctxB(bstate["pend"], True)
                    bstate["pend"] = None
                    normalize_B(T, bstate)
                    bgO.extend(outpB_pieces(T))
                while bgP:
                    bgP.popleft()()
                return ctxA_t

            def normalize_A(t, ctxA_t):
                # reciprocal_approx_fast mis-reads PSUM at partition offsets;
                # stage l into SBUF partition 0 first (plain DVE ops rebase
                # partitions correctly).
                lsA = sm.tile([1, 512], F32, name="lsA")
                nc.vector.tensor_copy(lsA[:], ctxA_t[64:65, :])
                lrA = sm.tile([1, 512], F32, name="lrA")
                nc.vector.reciprocal_approx_fast(lrA[:], lsA[:])
                lbA = sm.tile([HD, 512], F32, name="lbA")
                nc.gpsimd.partition_broadcast(lbA[:], lrA[0:1, :])
                nc.vector.tensor_tensor(
                    cT[0:HD, 512 * t : 512 * (t + 1)],
                    ctxA_t[0:HD, :],
                    lbA[:],
                    MULT,
                )

            def normalize_B(T, bstate):
                ctxB_t = bstate["ctxB"]
                lsB = sm.tile([1, 512], F32, name="lsB")
                nc.vector.tensor_copy(lsB[:], ctxB_t[64:65, :])
                lrB = sm.tile([1, 512], F32, name="lrB")
                nc.vector.reciprocal_approx_fast(lrB[:], lsB[:])
                lbB = sm.tile([HD, 512], F32, name="lbB")
                nc.gpsimd.partition_broadcast(lbB[:], lrB[0:1, :])
                nc.vector.tensor_tensor(
                    cT[HD:P, 512 * T : 512 * (T + 1)],
                    ctxB_t[0:HD, :],
                    lbB[:],
                    MULT,
                )

            # ---- main schedule ----
            bgP = deque()  # projection pieces: must complete within the tile
            bgO = deque()  # out-projection pieces: carry across tiles
            bstate = {}
            for p in proj_pieces(0):
                p()
            for t in range(NT):
                if t < NT - 1:
                    bgP.extend(proj_pieces(t + 1))
                ctxA_t = attn(t, bgP, bgO, bstate)
                normalize_A(t, ctxA_t)
                bgO.extend(outpA_pieces(t))
            while bgO:
                bgO.popleft()()

    nc.compile()
    return nc


def _host_inputs(x, W_query, W_key, W_value, W_out):
    bf = ml_dtypes.bfloat16
    x2 = np.asarray(x, np.float32).reshape(S, D)
    xT = np.ascontiguousarray(x2.T).astype(bf)  # [768, 4096]
    xt8 = np.ascontiguousarray(
        xT.reshape(KC, P, NT, 512).transpose(2, 1, 0, 3)
    ).reshape(8 * P, KC, 512)
    xb8 = []
    for par in range(2):
        xbT = np.ascontiguousarray(x2[par::2].T).astype(bf)  # [768, 2048]
        xb8.append(
            np.ascontiguousarray(
                xbT.reshape(KC, P, 4, 512).transpose(2, 1, 0, 3)
            ).reshape(4 * P, KC, 512)
        )
    ii, jj = np.arange(P)[:, None], np.arange(P)[None, :]
    ma = np.where(ii > jj, NEG, 0.0).astype(np.float32)
    jb = np.arange(HD)[None, :]
    mb = [
        np.where(ii > 2 * jb + par, NEG, 0.0).astype(np.float32)
        for par in range(2)
    ]
    ident = np.eye(P, dtype=bf)

    def wslice(w, h):
        return np.asarray(w, np.float32)[:, HD * h : HD * (h + 1)]

    in_maps = []
    for core in range(8):
        ha, hb, par = core, 8 + core // 2, core % 2
        wq = np.concatenate([wslice(W_query, ha), wslice(W_query, hb)], axis=1)
        wk = np.concatenate([wslice(W_key, ha), wslice(W_key, hb)], axis=1)
        wv = np.concatenate([wslice(W_value, ha), wslice(W_value, hb)], axis=1)
        wo = np.concatenate(
            [
                np.asarray(W_out, np.float32)[HD * ha : HD * (ha + 1), :],
                np.asarray(W_out, np.float32)[HD * hb : HD * (hb + 1), :],
            ],
            axis=0,
        )
        in_maps.append(
            {
                "xt": xt8,
                "xb": xb8[par],
                "wq": np.ascontiguousarray(
                    wq.astype(bf).reshape(KC, P, P).transpose(1, 0, 2)
                ),
                "wk": np.ascontiguousarray(
                    wk.astype(bf).reshape(KC, P, P).transpose(1, 0, 2)
                ),
                "wv": np.ascontiguousarray(
                    wv.astype(bf).reshape(KC, P, P).transpose(1, 0, 2)
                ),
                "wo": np.ascontiguousarray(wo.astype(bf)),
                "ma": ma,
                "mb": mb[par],
                "ident": ident,
            }
        )
    return in_maps


def run(x, W_query, W_key, W_value, W_out, b_out, trace=False):
    global _CACHED_NC
    if _CACHED_NC is None:
        _CACHED_NC = build_nc()
    nc = _CACHED_NC
    in_maps = _host_inputs(x, W_query, W_key, W_value, W_out)
    res = run_bass_kernel_spmd(nc, in_maps, core_ids=list(range(8)), trace=trace)
    out = np.zeros((S, D), dtype=np.float32)
    for core in range(8):
        out += np.asarray(res.results[core]["outA"], dtype=np.float32)
    for core in range(8):
        par = core % 2
        out[par::2] += np.asarray(res.results[core]["outB"], dtype=np.float32)
    out += np.asarray(b_out, np.float32)[None, :]
    return out, res


def kernel(x, W_query, W_key, W_value, W_out, b_out):
    out, _ = run(
        np.asarray(x, np.float32).reshape(S, D),
        np.asarray(W_query, np.float32),
        np.asarray(W_key, np.float32),
        np.asarray(W_value, np.float32),
        np.asarray(W_out, np.float32),
        np.asarray(b_out, np.float32),
    )
    return out.reshape(1, S, D)


# revision 20
# speedup vs baseline: 1.0952x; 1.0952x over previous
"""Causal multi-head attention (B=1, S=4096, D=768, H=12, d_head=64) on 8
Trainium2 NeuronCores.

Sharding: exact 1.5 heads per core. Slot A = head c (c = core id 0..7), full
causal attention over all 4096 queries. Slot B = head 8 + c//2 restricted to
query tokens of parity c%2 (2048 alternate tokens, full key range), so the 4
remaining heads are each split across two cores by query parity with zero
duplicated work and a uniform SPMD program (the parity lives in the data:
host-gathered xB rows and a parity-dependent boundary mask).

All matmul operands are bf16 (PSUM accumulation stays f32); the host supplies
x already transposed (and parity-gathered for slot B), so the device does no
x transposes at all. Per query tile the kernel interleaves next-tile QKV
projections and previous-tile out-projections into the attention block loop to
keep the PE busy (and at full clock) while the Scalar engine runs the exps.
Softmax denominators come free via ones-columns appended to V; normalization
uses reciprocal_approx_fast on DVE; out-proj PSUM->SBUF copies run on GpSimd.
Partial outputs are written bf16; the host sums them (the all-reduce of the
row-parallel out projection) and adds b_out.
"""

import sys

sys.path.insert(0, "/opt/trn_rl_repo")

from collections import deque

import ml_dtypes
import numpy as np

import concourse.bass as bass
import concourse.tile as tile
from concourse import bacc, mybir
from concourse.bass_utils import run_bass_kernel_spmd

S = 4096
D = 768
HD = 64
P = 128
KC = D // P  # 6 contraction chunks for the projections
NT = 8  # 512-token query tiles
NEG = -1e30

F32 = mybir.dt.float32
BF16 = mybir.dt.bfloat16
AF = mybir.ActivationFunctionType
ADD = mybir.AluOpType.add
MULT = mybir.AluOpType.mult

_CACHED_NC = None


def build_nc():
    nc = bacc.Bacc("TRN2", target_bir_lowering=False, debug=False, num_devices=8)

    xt_d = nc.declare_dram_parameter("xt", [8 * P, KC, 512], BF16, isOutput=False)
    xb_d = nc.declare_dram_parameter("xb", [4 * P, KC, 512], BF16, isOutput=False)
    wq_d = nc.declare_dram_parameter("wq", [P, KC, P], BF16, isOutput=False)
    wk_d = nc.declare_dram_parameter("wk", [P, KC, P], BF16, isOutput=False)
    wv_d = nc.declare_dram_parameter("wv", [P, KC, P], BF16, isOutput=False)
    wo_d = nc.declare_dram_parameter("wo", [P, D], BF16, isOutput=False)
    ma_d = nc.declare_dram_parameter("ma", [P, P], F32, isOutput=False)
    mb_d = nc.declare_dram_parameter("mb", [P, HD], F32, isOutput=False)
    id_d = nc.declare_dram_parameter("ident", [P, P], BF16, isOutput=False)
    outa_d = nc.declare_dram_parameter("outA", [S, D], BF16, isOutput=True)
    outb_d = nc.declare_dram_parameter("outB", [S // 2, D], BF16, isOutput=True)

    with tile.TileContext(nc) as tc:
        with (
            tc.tile_pool(name="const", bufs=1) as const,
            tc.tile_pool(name="big", bufs=1) as big,
            tc.tile_pool(name="pt", bufs=6) as ptp,
            tc.tile_pool(name="vt", bufs=2) as vtp,
            tc.tile_pool(name="osb", bufs=3) as osbp,
            tc.tile_pool(name="sm", bufs=2) as sm,
            tc.tile_pool(name="ps", bufs=4, space="PSUM") as ps,
            tc.tile_pool(name="ctxA", bufs=2, space="PSUM") as ctxAp,
            tc.tile_pool(name="ctxB", bufs=1, space="PSUM") as ctxBp,
            tc.tile_pool(name="tpp", bufs=1, space="PSUM") as tpp,
        ):
            # ---- constants ----
            # Split each load 4 ways by partition: one queue processes
            # descriptors serially (~150ns each), so a 128-partition DMA on a
            # single queue takes ~19us; 4 queues cut that to ~5us.
            def dma4(dst, src):
                for q4 in range(4):
                    nc.sync.dma_start(dst[32 * q4 : 32 * (q4 + 1)],
                                      src[32 * q4 : 32 * (q4 + 1)])

            identb = const.tile([P, P], BF16)
            ma_s = const.tile([P, P], F32)
            mb_s = const.tile([P, HD], F32)
            wq_s = const.tile([P, KC, P], BF16)
            wk_s = const.tile([P, KC, P], BF16)
            wv_s = const.tile([P, KC, P], BF16)
            wo_s = const.tile([P, D], BF16)

            # ---- persistent activations ----
            xTs = big.tile([P, NT, KC, 512], BF16)  # x^T, group-major
            xBs = big.tile([P, 4, KC, 512], BF16)  # x^T of slot-B tokens
            qT = big.tile([P, S], BF16)  # rows 0:64 qA^T, 64:128 qB^T (cols 0:2048)
            k2 = big.tile([P, S], BF16)  # rows 0:64 kA^T, 64:128 kB^T
            # v natural per 128-key block: cols 0:64 vA, 64 ones, 66:130 vB,
            # 130 ones (65/131 unused)
            vNat = big.tile([P, S // P, 132], BF16)
            cT = big.tile([P, S], BF16)  # rows 0:64 ctxA^T, 64:128 ctxB^T

            def dma_x(dst, src, g):
                for q4 in range(4):
                    nc.sync.dma_start(
                        dst[32 * q4 : 32 * (q4 + 1), g, :, :],
                        src[P * g + 32 * q4 : P * g + 32 * (q4 + 1), :, :],
                    )

            # Critical wave: everything tile group 0 needs, first in the
            # queues.
            dma4(identb, id_d)
            dma4(ma_s, ma_d)
            dma4(mb_s, mb_d)
            dma4(wq_s, wq_d)
            dma4(wk_s, wk_d)
            dma4(wv_s, wv_d)
            dma_x(xTs, xt_d, 0)
            dma_x(xBs, xb_d, 0)

            nc.gpsimd.memset(vNat[:, :, 64], 1.0)
            nc.gpsimd.memset(vNat[:, :, 130], 1.0)

            # ---- PE warmup: ramp the clock while DMAs stream in ----
            for _ in range(24):
                wps = ps.tile([P, 512], F32, name="ps", tag="ps")
                nc.tensor.matmul(
                    wps[:, 0:P], identb[:], identb[:], start=True, stop=True
                )

            # Second wave: later tile groups, roughly in first-use order.
            dma_x(xTs, xt_d, 1)
            dma4(wo_s, wo_d)
            for t in range(2, NT):
                dma_x(xTs, xt_d, t)
                if t % 2 == 0 and t // 2 < 4:
                    dma_x(xBs, xb_d, t // 2)

            # ---- projection pieces for tile group t ----
            def mk_projK(t):
                def f():
                    pp = ps.tile([P, 512], F32, name="ps", tag="ps")
                    for c in range(KC):
                        nc.tensor.matmul(
                            pp[:],
                            wk_s[:, c, :],
                            xTs[:, t, c, :],
                            start=(c == 0),
                            stop=(c == KC - 1),
                        )
                    nc.vector.tensor_copy(k2[:, 512 * t : 512 * (t + 1)], pp[:])

                return f

            def mk_projV(t):
                def f():
                    pp = ps.tile([P, 512], F32, name="ps", tag="ps")
                    for c in range(KC):
                        nc.tensor.matmul(
                            pp[:],
                            wv_s[:, c, :],
                            xTs[:, t, c, :],
                            start=(c == 0),
                            stop=(c == KC - 1),
                        )
                    vt_t = vtp.tile([P, 512], BF16, name="vt")
                    nc.vector.tensor_copy(vt_t[:], pp[:])
                    f.vt = vt_t

                return f

            def mk_projQ(t):
                def f():
                    pp = ps.tile([P, 512], F32, name="ps", tag="ps")
                    for c in range(KC):
                        nc.tensor.matmul(
                            pp[0:HD, :],
                            wq_s[:, c, 0:HD],
                            xTs[:, t, c, :],
                            start=(c == 0),
                            stop=(c == KC - 1),
                        )
                    if t % 2 == 0:
                        g = t // 2
                        for c in range(KC):
                            nc.tensor.matmul(
                                pp[HD:P, :],
                                wq_s[:, c, HD:P],
                                xBs[:, g, c, :],
                                start=(c == 0),
                                stop=(c == KC - 1),
                            )
                    nc.vector.tensor_copy(
                        qT[0:HD, 512 * t : 512 * (t + 1)], pp[0:HD, :]
                    )
                    if t % 2 == 0:
                        g = t // 2
                        nc.vector.tensor_copy(
                            qT[HD:P, 512 * g : 512 * (g + 1)], pp[HD:P, :]
                        )

                return f

            def mk_transV(t, projv):
                def f():
                    tp = tpp.tile([P, 4, P], BF16, name="tp")
                    for b in range(4):
                        nc.tensor.transpose(
                            tp[:, b, :],
                            projv.vt[:, P * b : P * (b + 1)],
                            identb[:],
                        )
                    nc.vector.tensor_copy(
                        vNat[:, 4 * t : 4 * t + 4, 0:HD], tp[:, :, 0:HD]
                    )
                    nc.vector.tensor_copy(
                        vNat[:, 4 * t : 4 * t + 4, 66:130], tp[:, :, HD:P]
                    )

                return f

            def proj_pieces(t):
                pv = mk_projV(t)
                return [mk_projK(t), pv, mk_projQ(t), mk_transV(t, pv)]

            # ---- out-projection piece for one 128-row block ----
            def mk_outp(st, is_b):
                def f():
                    osb_t = osbp.tile([P, D], BF16, name="osb")
                    crow = cT[HD:P, :] if is_b else cT[0:HD, :]
                    wrow = wo_s[HD:P, :] if is_b else wo_s[0:HD, :]
                    for h in range(2):
                        po = ps.tile([P, 512], F32, name="ps", tag="ps")
                        nc.tensor.matmul(
                            po[:, 0:384],
                            crow[:, P * st : P * (st + 1)],
                            wrow[:, 384 * h : 384 * (h + 1)],
                            start=True,
                            stop=True,
                        )
                        nc.vector.tensor_copy(
                            osb_t[:, 384 * h : 384 * (h + 1)], po[:, 0:384]
                        )
                    dst = outb_d if is_b else outa_d
                    nc.sync.dma_start(dst[P * st : P * (st + 1), :], osb_t[:])

                return f

            def outpA_pieces(t):
                return [mk_outp(4 * t + i, False) for i in range(4)]

            def outpB_pieces(T):
                return [mk_outp(4 * T + i, True) for i in range(4)]

            # ---- attention for tile t; pops bg pieces into PE slack ----
            # Slot B runs 512-wide query tiles (B-tile T spans A-tiles 2T and
            # 2T+1): key blocks 0..8T+3 during the even tile, the 4 diagonal
            # blocks during the odd tile.
            def attn(t, bgP, bgO, bstate):
                T = t // 2
                nkb = 4 * (t + 1)
                ctxA_t = ctxAp.tile([P, 512], F32, name="ctxA")
                if t % 2 == 0:
                    bstate["ctxB"] = ctxBp.tile([P, 512], F32, name="ctxB")
                    bstate["pend"] = None
                    b_list = list(range(0, 8 * T + 4))
                else:
                    b_list = list(range(8 * T + 4, 8 * T + 8))
                ctxB_t = bstate["ctxB"]
                pend_A = None  # (pA, r0A, kb)

                def issue_ctxA(p, last):
                    pA, r0A, kb = p
                    nc.tensor.matmul(
                        ctxA_t[0:65, r0A:512],
                        vNat[:, kb, 0:65],
                        pA[:, r0A:512],
                        start=(kb == 0),
                        stop=last,
                    )

                def issue_ctxB(p, last):
                    pB, r0B, kb = p
                    nc.tensor.matmul(
                        ctxB_t[0:65, r0B:512],
                        vNat[:, kb, 66:131],
                        pB[:, r0B:512],
                        start=(kb == 0),
                        stop=last,
                    )

                bi = 0
                for i in range(nkb):
                    kb = i
                    if (
                        t % 2 == 1
                        and bi == len(b_list)
                        and bstate["pend"] is not None
                    ):
                        # B-tile T complete: normalize now and queue its
                        # out-proj into the remaining PE slack.
                        issue_ctxB(bstate["pend"], True)
                        bstate["pend"] = None
                        normalize_B(T, bstate)
                        bgO.extend(outpB_pieces(T))
                    d = kb - 4 * t  # >= 0 on diagonal blocks
                    # --- slot A scores + exp
                    r0A = P * d if d >= 0 else 0
                    scA = ps.tile([P, 512], F32, name="ps", tag="ps")
                    nc.tensor.matmul(
                        scA[:, r0A:512],
                        k2[0:HD, P * kb : P * (kb + 1)],
                        qT[0:HD, 512 * t + r0A : 512 * (t + 1)],
                        start=True,
                        stop=True,
                    )
                    if d >= 0:
                        nc.vector.tensor_tensor(
                            scA[:, r0A : r0A + P],
                            scA[:, r0A : r0A + P],
                            ma_s[:],
                            ADD,
                        )
                    pA = ptp.tile([P, 512], BF16, name="pt", tag="pt")
                    nc.scalar.activation(
                        pA[:, r0A:512], scA[:, r0A:512], AF.Exp, scale=0.125
                    )
                    # --- slot B block (512-wide query tile T)
                    if bi < len(b_list):
                        kbb = b_list[bi]
                        bi += 1
                        dB = kbb - 8 * T
                        r0B = HD * dB if dB >= 0 else 0
                        scB = ps.tile([P, 512], F32, name="ps", tag="ps")
                        nc.tensor.matmul(
                            scB[:, r0B:512],
                            k2[HD:P, P * kbb : P * (kbb + 1)],
                            qT[HD:P, 512 * T + r0B : 512 * (T + 1)],
                            start=True,
                            stop=True,
                        )
                        if dB >= 0:
                            nc.vector.tensor_tensor(
                                scB[:, r0B : r0B + HD],
                                scB[:, r0B : r0B + HD],
                                mb_s[:],
                                ADD,
                            )
                        pB = ptp.tile([P, 512], BF16, name="pt", tag="pt")
                        nc.scalar.activation(
                            pB[:, r0B:512], scB[:, r0B:512], AF.Exp, scale=0.125
                        )
                        if bstate["pend"] is not None:
                            issue_ctxB(bstate["pend"], False)
                        bstate["pend"] = (pB, r0B, kbb)
                    # --- lagged ctx for slot A
                    if pend_A is not None:
                        issue_ctxA(pend_A, False)
                    pend_A = (pA, r0A, kb)
                    # --- background pieces: proj must finish this tile;
                    # out-proj pieces carry over without burst drains.
                    rem = nkb - i
                    if bgP:
                        npop = min(len(bgP), max(1, -(-len(bgP) // rem)))
                        for _ in range(npop):
                            bgP.popleft()()
                    elif bgO and i % 2 == 0:
                        bgO.popleft()()
                issue_ctxA(pend_A, True)
                if t % 2 == 1 and bstate["pend"] is not None:
                    issue_ctxB(bstate["pend"], True)
                    bstate["pend"] = None
                    normalize_B(T, bstate)
                    bgO.extend(outpB_pieces(T))
                while bgP:
                    bgP.popleft()()
                return ctxA_t

            def normalize_A(t, ctxA_t):
                # reciprocal_approx_fast mis-reads PSUM at partition offsets;
                # stage l into SBUF partition 0 first (plain DVE ops rebase
                # partitions correctly).
                lsA = sm.tile([1, 512], F32, name="lsA")
                nc.vector.tensor_copy(lsA[:], ctxA_t[64:65, :])
                lrA = sm.tile([1, 512], F32, name="lrA")
                nc.vector.reciprocal_approx_fast(lrA[:], lsA[:])
                lbA = sm.tile([HD, 512], F32, name="lbA")
                nc.gpsimd.partition_broadcast(lbA[:], lrA[0:1, :])
                nc.vector.tensor_tensor(
                    cT[0:HD, 512 * t : 512 * (t + 1)],
                    ctxA_t[0:HD, :],
                    lbA[:],
                    MULT,
                )

            def normalize_B(T, bstate):
                ctxB_t = bstate["ctxB"]
                lsB = sm.tile([1, 512], F32, name="lsB")
                nc.vector.tensor_copy(lsB[:], ctxB_t[64:65, :])
                lrB = sm.tile([1, 512], F32, name="lrB")
                nc.vector.reciprocal_approx_fast(lrB[:], lsB[:])
                lbB = sm.tile([HD, 512], F32, name="lbB")
                nc.gpsimd.partition_broadcast(lbB[:], lrB[0:1, :])
                nc.vector.tensor_tensor(
                    cT[HD:P, 512 * T : 512 * (T + 1)],
                    ctxB_t[0:HD, :],
                    lbB[:],
                    MULT,
                )

            # ---- main schedule ----
            bgP = deque()  # projection pieces: must complete within the tile
            bgO = deque()  # out-projection pieces: carry across tiles
            bstate = {}
            for p in proj_pieces(0):
                p()
            for t in range(NT):
                if t < NT - 1:
                    bgP.extend(proj_pieces(t + 1))
                ctxA_t = attn(t, bgP, bgO, bstate)
                normalize_A(t, ctxA_t)
                bgO.extend(outpA_pieces(t))
            while bgO:
                bgO.popleft()()

    nc.compile()
    return nc


def _host_inputs(x, W_query, W_key, W_value, W_out):
    bf = ml_dtypes.bfloat16
    x2 = np.asarray(x, np.float32).reshape(S, D)
    xT = np.ascontiguousarray(x2.T).astype(bf)  # [768, 4096]
    xt8 = np.ascontiguousarray(
        xT.reshape(KC, P, NT, 512).transpose(2, 1, 0, 3)
    ).reshape(8 * P, KC, 512)
    xb8 = []
    for par in range(2):
        xbT = np.ascontiguousarray(x2[par::2].T).astype(bf)  # [768, 2048]
        xb8.append(
            np.ascontiguousarray(
                xbT.reshape(KC, P, 4, 512).transpose(2, 1, 0, 3)
            ).reshape(4 * P, KC, 512)
        )
    ii, jj = np.arange(P)[:, None], np.arange(P)[None, :]
    ma = np.where(ii > jj, NEG, 0.0).astype(np.float32)
    jb = np.arange(HD)[None, :]
    mb = [
        np.where(ii > 2 * jb + par, NEG, 0.0).astype(np.float32)
        for par in range(2)
    ]
    ident = np.eye(P, dtype=bf)

    def wslice(w, h):
        return np.asarray(w, np.float32)[:, HD * h : HD * (h + 1)]

    in_maps = []
    for core in range(8):
        ha, hb, par = core, 8 + core // 2, core % 2
        wq = np.concatenate([wslice(W_query, ha), wslice(W_query, hb)], axis=1)
        wk = np.concatenate([wslice(W_key, ha), wslice(W_key, hb)], axis=1)
        wv = np.concatenate([wslice(W_value, ha), wslice(W_value, hb)], axis=1)
        wo = np.concatenate(
            [
                np.asarray(W_out, np.float32)[HD * ha : HD * (ha + 1), :],
                np.asarray(W_out, np.float32)[HD * hb : HD * (hb + 1), :],
            ],
            axis=0,
        )
        in_maps.append(
            {
                "xt": xt8,
                "xb": xb8[par],
                "wq": np.ascontiguousarray(
                    wq.astype(bf).reshape(KC, P, P).transpose(1, 0, 2)
                ),
                "wk": np.ascontiguousarray(
                    wk.astype(bf).reshape(KC, P, P).transpose(1, 0, 2)
                ),
                "wv": np.ascontiguousarray(
                    wv.astype(bf).reshape(KC, P, P).transpose(1, 0, 2)
                ),
                "wo": np.ascontiguousarray(wo.astype(bf)),
                "ma": ma,
                "mb": mb[par],
                "ident": ident,
            }
        )
    return in_maps


def run(x, W_query, W_key, W_value, W_out, b_out, trace=False):
    global _CACHED_NC
    if _CACHED_NC is None:
        _CACHED_NC = build_nc()
    nc = _CACHED_NC
    in_maps = _host_inputs(x, W_query, W_key, W_value, W_out)
    res = run_bass_kernel_spmd(nc, in_maps, core_ids=list(range(8)), trace=trace)
    out = np.zeros((S, D), dtype=np.float32)
    for core in range(8):
        out += np.asarray(res.results[core]["outA"], dtype=np.float32)
    for core in range(8):
        par = core % 2
        out[par::2] += np.asarray(res.results[core]["outB"], dtype=np.float32)
    out += np.asarray(b_out, np.float32)[None, :]
    return out, res


def kernel(x, W_query, W_key, W_value, W_out, b_out):
    out, _ = run(
        np.asarray(x, np.float32).reshape(S, D),
        np.asarray(W_query, np.float32),
        np.asarray(W_key, np.float32),
        np.asarray(W_value, np.float32),
        np.asarray(W_out, np.float32),
        np.asarray(b_out, np.float32),
    )
    return out.reshape(1, S, D)
